# revision 4
# baseline (speedup 1.0000x reference)
"""Trainium2 Bass kernel for the CrossLayer problem.

Math: reference computes, per row x (length D), with cur_0 = x:
    cur_{i+1} = sum(cur_i) * (w_i ⊙ x) + b_i + x        (i = 0..L-1)
Only the scalar s_i = sum(cur_i) couples elements, so with
    X   = sum(x)                  (per row)
    W_i = x · w_i                 (per row, i = 0..L-2)
    c_i = sum(b_i)
the recursion collapses to scalars:
    S_0 = X;  S_{i+1} = S_i * W_i + c_i + X
and the output is a single elementwise pass:
    out = S_{L-1} * (w_{L-1} ⊙ x) + b_{L-1} + x

Kernel layout (per core, pure data parallel over batch):
  - rows on partitions, 16 tiles of (128, 1024) f32
  - PE transposes each tile chunk (128x128) so the tensor engine can
    compute [X, W_0, W_1, W_2] = Wpk^T @ x^T with one PSUM accumulation
  - tiny PE transpose puts the 4 dots back row-major; tensor_scalar ops
    run the scalar recursion
  - final output: tensor_mul (w3 ⊙ x) + fused scalar_tensor_tensor
    (S3 * w3x + x); the general-b path adds b3 with one more pass
"""

import os
import numpy as np

B, D, L = 16384, 1024, 4
N_CORES = 8
RPC = B // N_CORES          # rows per core
P = 128                     # partitions
N_TILES = RPC // P          # 16
N_CHUNKS = D // P           # 8

_built = {}


def _build_nc(b_zero: bool):
    import concourse.bass as bass
    import concourse.bacc as bacc
    import concourse.mybir as mybir
    from concourse import tile

    f32 = mybir.dt.float32
    Alu = mybir.AluOpType
    Act = mybir.ActivationFunctionType

    # Bacc (not raw Bass): its compile() legalizes semaphore waits — TRN2
    # matmuls encode at most one sync wait (walrus S3_LW struct).
    nc = bacc.Bacc(
        "TRN2", target_bir_lowering=False, debug=False, num_devices=N_CORES
    )
    x_d = nc.dram_tensor("x", [RPC, D], f32, kind="ExternalInput")
    wpk_d = nc.dram_tensor("wpk", [P, N_CHUNKS * 4], f32, kind="ExternalInput")
    w3bc_d = nc.dram_tensor("w3bc", [P, D], f32, kind="ExternalInput")
    ident_d = nc.dram_tensor("ident", [P, P], f32, kind="ExternalInput")
    if not b_zero:
        cvec_d = nc.dram_tensor("cvec", [P, 4], f32, kind="ExternalInput")
        b3bc_d = nc.dram_tensor("b3bc", [P, D], f32, kind="ExternalInput")
    out_d = nc.dram_tensor("out", [RPC, D], f32, kind="ExternalOutput")

    with tile.TileContext(nc) as tc:
        with (
            tc.tile_pool(name="consts", bufs=1) as consts,
            tc.tile_pool(name="xin", bufs=4) as xin_pool,
            tc.tile_pool(name="mid", bufs=3) as mid_pool,
            tc.tile_pool(name="outp", bufs=4) as out_pool,
            tc.tile_pool(name="small", bufs=3) as small_pool,
            tc.tile_pool(name="ps_t", bufs=2, space=bass.MemorySpace.PSUM) as ps_t,
            tc.tile_pool(name="ps_d", bufs=2, space=bass.MemorySpace.PSUM) as ps_d,
            tc.tile_pool(name="ps_s", bufs=2, space=bass.MemorySpace.PSUM) as ps_s,
        ):
            wpk = consts.tile([P, N_CHUNKS * 4], f32)
            nc.sync.dma_start(wpk[:], wpk_d[:])
            w3bc = consts.tile([P, D], f32)
            nc.sync.dma_start(w3bc[:], w3bc_d[:])
            ident = consts.tile([P, P], f32)
            nc.sync.dma_start(ident[:], ident_d[:])
            if not b_zero:
                cvec = consts.tile([P, 4], f32)
                nc.sync.dma_start(cvec[:], cvec_d[:])
                b3bc = consts.tile([P, D], f32)
                nc.sync.dma_start(b3bc[:], b3bc_d[:])

            # Prologue: absorb each const-DMA completion into one engine
            # observation up front. The LDWEIGHTS side of a matmul encodes
            # only one sync wait, so steady-state matmuls must not need two
            # fresh semaphore waits (walrus: "Too many sync wait commands").
            prol1 = ps_s.tile([P, P], f32, name="prol1", tag="dT_ps")
            nc.tensor.transpose(prol1[:], ident[:], ident[:])
            prol2 = ps_d.tile([4, P], f32, name="prol2", tag="dots_ps")
            nc.tensor.matmul(prol2[:], wpk[:, 0:4], ident[:], start=True, stop=True)
            prolv = small_pool.tile([P, 1], f32, name="prolv")
            nc.vector.tensor_copy(prolv[:], w3bc[:, 0:1])
            if not b_zero:
                prolc = small_pool.tile([P, 1], f32, name="prolc")
                nc.vector.tensor_copy(prolc[:], cvec[:, 0:1])
                prolb = small_pool.tile([P, 1], f32, name="prolb")
                nc.vector.tensor_copy(prolb[:], b3bc[:, 0:1])

            for t in range(N_TILES):
                xt = xin_pool.tile([P, D], f32, name="xt")
                nc.sync.dma_start(xt[:], x_d[t * P:(t + 1) * P, :])

                # x^T, chunk by chunk: xT[:, c*128+r] holds x[r, c*128+p]
                xT_ps = ps_t.tile([P, D], f32, name="xT_ps")
                for c in range(N_CHUNKS):
                    nc.tensor.transpose(
                        xT_ps[:, c * P:(c + 1) * P],
                        xt[:, c * P:(c + 1) * P],
                        ident[:],
                    )
                xT = mid_pool.tile([P, D], f32, name="xT")
                nc.scalar.copy(xT[:], xT_ps[:])

                # dots[i, r] = [X, W0, W1, W2][r], accumulated over chunks
                dots_ps = ps_d.tile([4, P], f32, name="dots_ps")
                for c in range(N_CHUNKS):
                    nc.tensor.matmul(
                        dots_ps[:],
                        wpk[:, c * 4:(c + 1) * 4],
                        xT[:, c * P:(c + 1) * P],
                        start=(c == 0),
                        stop=(c == N_CHUNKS - 1),
                    )
                dots = small_pool.tile([4, P], f32, name="dots")
                nc.scalar.copy(dots[:], dots_ps[:])

                # back to row-major: dT[r, i]
                dT_ps = ps_s.tile([P, 4], f32, name="dT_ps")
                nc.tensor.transpose(dT_ps[:], dots[:], ident[0:4, 0:4])
                dT = small_pool.tile([P, 4], f32, name="dT")
                nc.scalar.copy(dT[:], dT_ps[:])

                # scalar recursion S_{i+1} = S_i * W_i + (X + c_i)
                svec = small_pool.tile([P, 4], f32, name="svec")
                X = dT[:, 0:1]
                if b_zero:
                    addends = [X, X, X]
                else:
                    avec = small_pool.tile([P, 4], f32, name="avec")
                    for i in range(3):
                        nc.vector.tensor_scalar_add(
                            avec[:, i:i + 1], X, cvec[:, i:i + 1]
                        )
                    addends = [avec[:, 0:1], avec[:, 1:2], avec[:, 2:3]]
                s_prev = X
                for i in range(3):
                    nc.vector.tensor_scalar(
                        svec[:, i:i + 1],
                        s_prev,
                        dT[:, i + 1:i + 2],
                        addends[i],
                        Alu.mult,
                        Alu.add,
                    )
                    s_prev = svec[:, i:i + 1]
                S3 = svec[:, 2:3]

                # out = S3 * (w3 ⊙ x) + x (+ b3)
                w3x = mid_pool.tile([P, D], f32, name="w3x")
                nc.vector.tensor_mul(w3x[:], xt[:], w3bc[:])
                out_sb = out_pool.tile([P, D], f32, name="out_sb")
                nc.vector.scalar_tensor_tensor(
                    out_sb[:], w3x[:], S3, xt[:], Alu.mult, Alu.add
                )
                if not b_zero:
                    out2 = out_pool.tile([P, D], f32, name="out2")
                    nc.vector.tensor_add(out2[:], out_sb[:], b3bc[:])
                    out_sb = out2
                nc.sync.dma_start(out_d[t * P:(t + 1) * P, :], out_sb[:])
    nc.compile()
    return nc


def _get_nc(b_zero: bool):
    if b_zero not in _built:
        _built[b_zero] = _build_nc(b_zero)
    return _built[b_zero]


def _host_prep(w, b, b_zero):
    # Wpk[p, c*4+i] packs column i of [ones, w0, w1, w2] for D-chunk c
    M = np.empty((D, 4), dtype=np.float32)
    M[:, 0] = 1.0
    M[:, 1] = w[0]
    M[:, 2] = w[1]
    M[:, 3] = w[2]
    wpk = np.ascontiguousarray(
        M.reshape(N_CHUNKS, P, 4).transpose(1, 0, 2).reshape(P, N_CHUNKS * 4)
    )
    w3bc = np.ascontiguousarray(np.broadcast_to(w[3], (P, D)).astype(np.float32))
    ident = np.eye(P, dtype=np.float32)
    extras = {}
    if not b_zero:
        c = b.sum(axis=1).astype(np.float32)  # (L,)
        extras["cvec"] = np.ascontiguousarray(np.broadcast_to(c, (P, L)))
        extras["b3bc"] = np.ascontiguousarray(
            np.broadcast_to(b[3], (P, D)).astype(np.float32)
        )
    return wpk, w3bc, ident, extras


def kernel(inputs, w, b):
    from concourse.bass_utils import run_bass_kernel_spmd

    x = np.ascontiguousarray(np.asarray(inputs, dtype=np.float32).reshape(B, D))
    w = np.asarray(w, dtype=np.float32)
    b = np.asarray(b, dtype=np.float32)
    b_zero = not b.any()

    nc = _get_nc(b_zero)
    wpk, w3bc, ident, extras = _host_prep(w, b, b_zero)

    in_maps = []
    for i in range(N_CORES):
        m = {
            "x": x[i * RPC:(i + 1) * RPC],
            "wpk": wpk,
            "w3bc": w3bc,
            "ident": ident,
        }
        m.update(extras)
        in_maps.append(m)

    trace = bool(int(os.environ.get("KERNEL_TRACE", "0")))
    kwargs = {}
    if trace:
        kwargs = {"trace": True, "trace_cores": [0]}
    res = run_bass_kernel_spmd(nc, in_maps, core_ids=list(range(N_CORES)), **kwargs)
    if trace:
        kernel.last_results = res
    return np.concatenate([r["out"] for r in res.results], axis=0)
